# revision 1
# baseline (speedup 1.0000x reference)
"""Trainium2 Bass kernel for nn_AttentionBlock (B=8, H=W=32, C=512, 8 heads).

Strategy: data-parallel over batch -- each of the 8 NeuronCores processes one
batch element end-to-end (no collectives).  Per core:

  x [T=1024, C=512] -> qkv -> per-head attention (T x T softmax) -> out proj.

All matmuls run as float32r (TF32) on the PE at full rate with fp32 PSUM
accumulation.  Softmax is computed in the S^T ([s, t]) layout so the
softmax reduction axis lands on the PSUM partition axis, where the
denominators come for free from a ones-column appended to V during the PV
matmul.  No max-subtraction is needed: logits are ~N(0, 1) by construction
(exp is evaluated with the 1/8 scale folded into the ScalarE activation).
"""

import math
import os
from contextlib import ExitStack

import numpy as np

import concourse.bass as bass
import concourse.mybir as mybir
import concourse.tile as tile
from concourse import bacc

T = 1024          # tokens per batch element (32*32)
C = 512           # channels
HEADS = 8
HC = C // HEADS   # 64
P = 128           # partitions
NT = T // P       # 8 t-tiles
NCT = C // P      # 4 c-tiles
CHUNK = 512       # moving-operand chunk (fp32 max, = one PSUM bank)
NCH = T // CHUNK  # 2 chunks
F32 = mybir.dt.float32
F32R = mybir.dt.float32r
EXP_SCALE = 1.0 / math.sqrt(HC)  # (1/sqrt(sqrt(hc)))^2 applied to q·k
ACT_GROUP = 2     # S^T chunks per ScalarE exp call (2 PSUM banks)


def tf32_round(a: np.ndarray) -> np.ndarray:
    """Round fp32 -> tf32 (10-bit mantissa) with round-to-nearest-even."""
    bits = a.astype(np.float32).view(np.uint32)
    round_bit = np.uint32(1 << 12)
    lsb = (bits >> np.uint32(13)) & np.uint32(1)
    bits = bits + (round_bit - np.uint32(1)) + lsb
    bits &= np.uint32(0xFFFFE000)
    return bits.view(np.float32)


def build_program(debug_dumps: bool = False):
    nc = bacc.Bacc("TRN2", num_devices=8, debug=False)

    x_d = nc.dram_tensor("x", [T, C], F32, kind="ExternalInput")
    wqkv_d = nc.dram_tensor("qkv_w", [C, 3 * C], F32R, kind="ExternalInput")
    wout_d = nc.dram_tensor("out_w", [C, C], F32R, kind="ExternalInput")
    qkb_d = nc.dram_tensor("qk_b", [2 * C], F32, kind="ExternalInput")
    ob_d = nc.dram_tensor("out_b", [C], F32, kind="ExternalInput")
    out_d = nc.dram_tensor("out", [T, C], F32, kind="ExternalOutput")
    dbg = {}
    if debug_dumps:
        for nm, shp in [
            ("dbg_xT", [P, T]), ("dbg_qT", [P, T]), ("dbg_kT", [P, T]),
            ("dbg_v", [P, HEADS * (HC + 1)]), ("dbg_ex", [P, 3 * CHUNK]),
            ("dbg_pv", [HC + 1, CHUNK]), ("dbg_an", [P, T]),
            ("dbg_recip", [1, T]), ("dbg_bcast", [HC, T]),
        ]:
            dbg[nm] = nc.dram_tensor(nm, shp, F32, kind="ExternalOutput")

    with tile.TileContext(nc) as tc, ExitStack() as ctx:
        from concourse.masks import make_identity

        # ---------------- SBUF pools (whole-kernel lifetime) ----------------
        const = ctx.enter_context(tc.tile_pool(name="const", bufs=1))
        persist = ctx.enter_context(tc.tile_pool(name="persist", bufs=1))
        workp = ctx.enter_context(tc.tile_pool(name="workp", bufs=1))
        xload_cm = tc.tile_pool(name="xload", bufs=1)
        xload = xload_cm.__enter__()

        # x tiles first: they gate the transpose pipeline
        xts = []
        for i in range(NT):
            xt_in = xload.tile([P, C], F32, tag=f"x_in{i}", name=f"x_in{i}")
            nc.sync.dma_start(xt_in[:], x_d.ap()[i * P:(i + 1) * P, :])
            xts.append(xt_in)

        # weights straight into fp32r tiles (host pre-rounds the data);
        # v columns land first so the v matmuls can start early
        wq = []  # [c-tile][128, 1536]
        for m in range(NCT):
            t_ = persist.tile([P, 3 * C], F32R, tag=f"wq{m}", name=f"wq{m}")
            nc.gpsimd.dma_start(t_[:, 2 * C:3 * C],
                                wqkv_d.ap()[m * P:(m + 1) * P, 2 * C:3 * C])
            wq.append(t_)
        for m in range(NCT):
            nc.gpsimd.dma_start(wq[m][:, 0:2 * C],
                                wqkv_d.ap()[m * P:(m + 1) * P, 0:2 * C])
        identity = const.tile([P, P], F32, tag="ident", name="ident")
        make_identity(nc, identity[:])

        ones8 = const.tile([P, HEADS, 1], F32, tag="ones8", name="ones8")
        nc.gpsimd.memset(ones8[:], 1.0)

        # bias tiles (single gather DMA each); column m = bias[128m:128m+128]
        qkb_all = const.tile([P, 2 * C // P], F32, tag="qkball", name="qkb_all")
        nc.gpsimd.dma_start(
            qkb_all[:], qkb_d.ap().rearrange("(m p) -> p m", p=P)
        )
        qkb_t = [qkb_all[:, m:m + 1] for m in range(2 * C // P)]

        xT = [xload.tile([P, T], F32R, tag=f"xT{m}", name=f"xT{m}") for m in range(NCT)]
        VAW = HEADS * (HC + 1) + (P - HC - 1)  # 128-wide lhsT reads stay in-tile
        vaug = [persist.tile([P, VAW], F32R, tag=f"va{i}", name=f"va{i}") for i in range(NT)]
        qkT = [persist.tile([P, T], F32R, tag=f"qk{m}", name=f"qk{m}") for m in range(C // P)]
        # per-head zero-padded k^T: even heads use rows 0:64 (zeros below),
        # odd heads rows 64:128 (zeros above) so K=128 S^T matmuls pair with
        # the full q^T tile rows directly.
        kTz = [persist.tile([P, T], F32R, tag=f"kz{h}", name=f"kz{h}") for h in range(HEADS)]
        anorm = [persist.tile([P, T], F32R, tag=f"an{m}", name=f"an{m}") for m in range(NCT)]

        # ================= phase 1: x^T, v, q^T/k^T =================
        with tc.tile_pool(name="ps1", bufs=2, space="PSUM") as ps1:
            # x PE transpose; xT[m] = x^T rows [128m,128m+128) [c, t]
            for i in range(NT):
                xt_in = xts[i]
                ps_tr = ps1.tile([P, C], F32, tag="tr", name="ps_tr")
                for m in range(NCT):
                    nc.tensor.transpose(
                        ps_tr[:, m * P:(m + 1) * P],
                        xt_in[:, m * P:(m + 1) * P],
                        identity[:],
                    )
                for m in range(NCT):
                    nc.vector.tensor_copy(
                        xT[m][:, i * P:(i + 1) * P], ps_tr[:, m * P:(m + 1) * P]
                    )

            # v = x @ Wv; vaug[i]: [128(t), 8, 65], [:, h, 64] = 1.0
            for i in range(NT):
                ps_v = ps1.tile([P, C], F32, tag="v", name="ps_v")
                for m in range(NCT):
                    nc.tensor.matmul(
                        ps_v[:],
                        xT[m][:, i * P:(i + 1) * P],
                        wq[m][:, 2 * C:3 * C],
                        start=(m == 0),
                        stop=(m == NCT - 1),
                    )
                va3 = vaug[i][:, 0:HEADS * (HC + 1)].rearrange(
                    "p (h d) -> p h d", d=HC + 1)
                nc.vector.tensor_copy(
                    va3[:, :, 0:HC],
                    ps_v[:].rearrange("p (h d) -> p h d", h=HEADS),
                )
                nc.vector.tensor_copy(va3[:, :, HC:HC + 1], ones8[:])
                nc.vector.tensor_scalar_mul(
                    vaug[i][:, HEADS * (HC + 1):VAW],
                    ps_v[:, 0:VAW - HEADS * (HC + 1)], 0.0)

            # zero-fill the padding halves of kTz
            for h in range(HEADS):
                zlo = 0 if h % 2 == 1 else HC
                nc.vector.tensor_scalar_mul(
                    kTz[h][zlo:zlo + HC, :], wq[0][0:HC, 0:T], 0.0)
            # q^T/k^T: interleave q/k tile order so head-pair p's attention
            # can start as soon as qkT[p] and kTz[2p..2p+1] exist.
            for m in [0, 4, 1, 5, 2, 6, 3, 7]:
                for j in range(NCH):
                    ps_qk = ps1.tile([P, CHUNK], F32, tag="qk", name="ps_qk")
                    for cc in range(NCT):
                        nc.tensor.matmul(
                            ps_qk[:],
                            wq[cc][:, m * P:(m + 1) * P],
                            xT[cc][:, j * CHUNK:(j + 1) * CHUNK],
                            start=(cc == 0),
                            stop=(cc == NCT - 1),
                        )
                    js = slice(j * CHUNK, (j + 1) * CHUNK)
                    if m < NCT:
                        nc.vector.tensor_scalar_add(
                            qkT[m][:, js], ps_qk[:], qkb_t[m][:]
                        )
                    else:
                        hh = 2 * (m - NCT)
                        nc.vector.tensor_scalar_add(
                            kTz[hh][0:HC, js], ps_qk[0:HC, :],
                            qkb_t[m][0:HC],
                        )
                        nc.vector.tensor_scalar_add(
                            kTz[hh + 1][HC:P, js], ps_qk[HC:P, :],
                            qkb_t[m][HC:P],
                        )

            # out-proj weights + bias: only needed in phase 3; load last
            wo = []  # [c-tile][128, 512]
            for m in range(NCT):
                t_ = persist.tile([P, C], F32R, tag=f"wo{m}", name=f"wo{m}")
                nc.sync.dma_start(t_[:], wout_d.ap()[m * P:(m + 1) * P, :])
                wo.append(t_)
            ob_all = const.tile([P, NCT], F32, tag="oball", name="ob_all")
            nc.sync.dma_start(ob_all[:], ob_d.ap().rearrange("(m p) -> p m", p=P))
            ob_t = [ob_all[:, m:m + 1] for m in range(NCT)]

        dbgp = ctx.enter_context(tc.tile_pool(name="dbgp", bufs=1)) if debug_dumps else None
        if debug_dumps:
            cp = dbgp.tile([P, T], F32, tag="dbg", name="dbgcp")
            nc.vector.tensor_copy(cp[:], xT[0][:].bitcast(F32))
            nc.sync.dma_start(dbg["dbg_xT"].ap(), cp[:])
        xload_cm.__exit__(None, None, None)
        if debug_dumps:
            cp2 = dbgp.tile([P, T], F32, tag="dbg", name="dbgcp2")
            nc.vector.tensor_copy(cp2[:], qkT[0][:].bitcast(F32))
            nc.sync.dma_start(dbg["dbg_qT"].ap(), cp2[:])
            cp3 = dbgp.tile([P, T], F32, tag="dbg", name="dbgcp3")
            nc.vector.tensor_copy(cp3[:], kTz[0][:].bitcast(F32))
            nc.sync.dma_start(dbg["dbg_kT"].ap(), cp3[:])
            cp4 = dbgp.tile([P, HEADS * (HC + 1)], F32, tag="dbg", name="dbgcp4")
            nc.vector.tensor_copy(
                cp4[:], vaug[0][:, 0:HEADS * (HC + 1)].bitcast(F32)
            )
            nc.sync.dma_start(dbg["dbg_v"].ap(), cp4[:])

        # ================= phase 2: attention =================
        # Per head: stream the 8 S^T s-tiles through PSUM -> exp into a
        # full-head expS buffer (ACT-bound stretch).  The PREVIOUS head's 16
        # PV matmuls are interleaved two-per-slot between the S^T fills: they
        # are wait-free (their exp inputs completed last head), so they fill
        # the PE gaps and keep the HAM clock warm.
        def emit_pv_slice(ph, ext, ppv, chunks):
            for c in chunks:
                ssi, j = c // NCH, c % NCH
                nc.tensor.matmul(
                    ppv[j][:],
                    vaug[ssi][:, ph * (HC + 1): ph * (HC + 1) + P],
                    ext[:, c * CHUNK:(c + 1) * CHUNK],
                    start=(ssi == NT - 1),
                    stop=(ssi == 0),
                )

        def emit_normalize(ph, ppv):
            aoff = (ph % 2) * HC
            am = ph // 2
            dtmp = workp.tile([1, T], F32, tag="dtmp", name="dtmp")
            for j in range(NCH):
                nc.vector.tensor_copy(
                    dtmp[:, j * CHUNK:(j + 1) * CHUNK],
                    ppv[j][HC:HC + 1, :],
                )
            recip = workp.tile([1, T], F32, tag="recip", name="recip")
            nc.vector.reciprocal_approx_fast(recip[:], dtmp[:])
            bcast = workp.tile([HC, T], F32, tag="bcast", name="bcast")
            nc.gpsimd.partition_broadcast(bcast[:], recip[:], channels=HC)
            if debug_dumps and ph == 0:
                nc.sync.dma_start(dbg["dbg_recip"].ap(), recip[:])
                nc.sync.dma_start(dbg["dbg_bcast"].ap(), bcast[:])
            for j in range(NCH):
                nc.vector.tensor_tensor(
                    anorm[am][aoff:aoff + HC, j * CHUNK:(j + 1) * CHUNK],
                    ppv[j][0:HC, :],
                    bcast[:, j * CHUNK:(j + 1) * CHUNK],
                    op=mybir.AluOpType.mult,
                )

        # chunk stream per head: 16 chunks c=(si, j), ScalarE exp groups of 3
        CGRP = [list(range(g, min(g + 2, 2 * NT))) for g in range(0, 2 * NT, 2)]
        # prev-head PV chunks interleaved per group slot, reversed order
        PV_SLICES = []
        rc = list(reversed(range(2 * NT)))
        kk = 0
        for g in range(len(CGRP)):
            take = 2 if g < len(CGRP) - 1 else len(rc) - kk
            PV_SLICES.append(rc[kk:kk + take])
            kk += take

        with (
            tc.tile_pool(name="expsp", bufs=2) as expsp,
            tc.tile_pool(name="ps_st", bufs=2, space="PSUM") as ps_st,
            tc.tile_pool(name="ps_pv", bufs=2, space="PSUM") as ps_pv,
        ):
            prev = None  # (head, exh, pv tiles) whose PV burst is pending
            for h in range(HEADS):
                qm = h // 2
                pv_ps = [ps_pv.tile([P, CHUNK], F32, tag=f"pv{j}",
                                    name=f"pv{j}") for j in range(NCH)]
                exh = expsp.tile([P, 2 * NT * CHUNK], F32R, tag="exh", name="exh")
                for g, grp in enumerate(CGRP):
                    gw = len(grp)
                    st_ps = ps_st.tile([P, 2 * CHUNK], F32, tag="st", name="st")
                    for b, c in enumerate(grp):
                        ssi, j = c // NCH, c % NCH
                        nc.tensor.matmul(
                            st_ps[:, b * CHUNK:(b + 1) * CHUNK],
                            kTz[h][:, ssi * P:(ssi + 1) * P],
                            qkT[qm][:, j * CHUNK:(j + 1) * CHUNK],
                            start=True,
                            stop=True,
                        )
                    nc.scalar.activation(
                        exh[:, grp[0] * CHUNK:(grp[-1] + 1) * CHUNK],
                        st_ps[:, 0:gw * CHUNK],
                        mybir.ActivationFunctionType.Exp,
                        scale=EXP_SCALE,
                    )
                    if prev is not None:
                        emit_pv_slice(prev[0], prev[1], prev[2], PV_SLICES[g])
                if prev is not None:
                    emit_normalize(prev[0], prev[2])
                if debug_dumps and h == 0:
                    cp5 = dbgp.tile([P, 3 * CHUNK], F32, tag="dbg", name="dbgcp5")
                    nc.vector.tensor_copy(cp5[:, 0:T], exh[:, 0:T].bitcast(F32))
                    nc.vector.tensor_copy(
                        cp5[:, T:3 * CHUNK], exh[:, T:T + CHUNK].bitcast(F32))
                    nc.sync.dma_start(dbg["dbg_ex"].ap(), cp5[:])
                prev = (h, exh, pv_ps)
            # final head: straight burst + normalize
            emit_pv_slice(prev[0], prev[1], prev[2], list(reversed(range(2 * NT))))
            if debug_dumps:
                cp6 = dbgp.tile([HC + 1, CHUNK], F32, tag="dbg", name="dbgcp6")
                nc.vector.tensor_copy(cp6[:], prev[2][0][:])
                nc.sync.dma_start(dbg["dbg_pv"].ap(), cp6[:])
            emit_normalize(prev[0], prev[2])

        if debug_dumps:
            cp7 = dbgp.tile([P, T], F32, tag="dbg", name="dbgcp7")
            nc.vector.tensor_copy(cp7[:], anorm[0][:].bitcast(F32))
            nc.sync.dma_start(dbg["dbg_an"].ap(), cp7[:])

        # ================= phase 3: out projection + transpose =================
        with (
            tc.tile_pool(name="otp", bufs=1) as otp,
            tc.tile_pool(name="ps3", bufs=2, space="PSUM") as ps3,
        ):
            outT = []  # [e-tile][128, 1024] fp32
            for e in range(NCT):
                ps_o = ps3.tile([P, T], F32, tag="o", name="ps_o")
                for j in range(NCH):
                    for cc in range(NCT):
                        nc.tensor.matmul(
                            ps_o[:, j * CHUNK:(j + 1) * CHUNK],
                            wo[cc][:, e * P:(e + 1) * P],
                            anorm[cc][:, j * CHUNK:(j + 1) * CHUNK],
                            start=(cc == 0),
                            stop=(cc == NCT - 1),
                        )
                ot = otp.tile([P, T], F32, tag=f"ot{e}", name=f"ot{e}")
                nc.vector.tensor_scalar_add(ot[:], ps_o[:], ob_t[e][:])
                outT.append(ot)

            for i in range(NT):
                ps_tr = ps3.tile([P, C], F32, tag="tr2", name="ps_tr2")
                for e in range(NCT):
                    nc.tensor.transpose(
                        ps_tr[:, e * P:(e + 1) * P],
                        outT[e][:, i * P:(i + 1) * P],
                        identity[:],
                    )
                of = workp.tile([P, C], F32, tag="of", name="of")
                nc.scalar.copy(of[:], ps_tr[:])
                nc.sync.dma_start(out_d.ap()[i * P:(i + 1) * P, :], of[:])

    nc.compile()
    return nc


_CACHED_NC = None


def _get_nc():
    global _CACHED_NC
    if _CACHED_NC is None:
        _CACHED_NC = build_program()
    return _CACHED_NC


def kernel(x, qkv_w, qkv_b, out_w, out_b):
    """Full inputs in, full output out.  Shards batch across 8 NeuronCores."""
    from concourse.bass_utils import run_bass_kernel_spmd

    x = np.asarray(x)
    B, H, W, Cc = x.shape
    assert (B, H, W, Cc) == (8, 32, 32, C)
    x2 = np.ascontiguousarray(x.reshape(B, T, C).astype(np.float32))
    wq2 = np.asarray(qkv_w).reshape(C, 3 * C).astype(np.float32)
    wo2 = np.asarray(out_w).reshape(C, C).astype(np.float32)
    qkv_b = np.asarray(qkv_b).astype(np.float32)
    out_b = np.asarray(out_b).astype(np.float32)

    # host-side prep: tf32-round the weights (device loads them as float32r),
    # fold the v-bias through the output projection (exact: A_norm += b_v
    # shifts out by b_v @ W_out).
    wq_r = tf32_round(wq2)
    wo_r = tf32_round(wo2)
    b_v = qkv_b[2 * C:3 * C]
    ob_eff = (
        out_b.astype(np.float64) + b_v.astype(np.float64) @ wo_r.astype(np.float64)
    ).astype(np.float32)
    qkb = np.ascontiguousarray(qkv_b[0:2 * C])

    nc = _get_nc()
    in_maps = [
        {
            "x": np.ascontiguousarray(x2[b]),
            "qkv_w": np.ascontiguousarray(wq_r),
            "out_w": np.ascontiguousarray(wo_r),
            "qk_b": qkb,
            "out_b": ob_eff,
        }
        for b in range(B)
    ]
    trace = bool(int(os.environ.get("KERNEL_TRACE", "0")))
    res = run_bass_kernel_spmd(nc, in_maps, core_ids=list(range(B)), trace=trace)
    if trace and res.exec_time_ns is not None:
        print(f"HW exec time: {res.exec_time_ns} ns")
    kernel.last_results = res
    out = np.stack([res.results[b]["out"] for b in range(B)], axis=0)
    return out.reshape(B, H, W, Cc)


kernel.last_results = None



# revision 11
# speedup vs baseline: 1.2176x; 1.2176x over previous
"""Trainium2 Bass kernel for nn_AttentionBlock (B=8, H=W=32, C=512, 8 heads).

Strategy: data-parallel over batch -- each of the 8 NeuronCores processes one
batch element end-to-end (no collectives).  Per core:

  x [T=1024, C=512] -> qkv -> per-head attention (T x T softmax) -> out proj.

v2 design (ACT-bound pipeline):
  * all matmul data is bf16 (host pre-casts); PSUM accumulation stays fp32.
  * S^T = k^T q is computed with K=64 ROW-TILED matmul pairs: head 2p on PE
    row-groups 0-1 (SBUF partitions 0:64), head 2p+1 on row-groups 2-3
    (partitions 64:128).  The two matmuls run concurrently in the array, so
    a head-pair s-tile costs ~2x512 columns instead of 4x512.
  * softmax exp runs on ScalarE (the only exp engine) in N=1024 calls and is
    the phase-2 bottleneck (~73us); everything else (QKV projection, PV,
    out-projection) is woven into the PE gaps between exp calls.
  * denominators come free from a ones-column appended to V (PV row 64);
    normalization = DVE multiply by a GPSIMD-broadcast reciprocal.
  * out-projection uses anorm tiles as lhsT so the output lands directly in
    [t, c] layout -- no output transpose.  Output is bf16; host casts back.
  * no max-subtraction: logits are ~N(0,1) by construction (1/8 scale is
    folded into the ScalarE exp activation).
"""

import math
import os
from contextlib import ExitStack

import numpy as np

import concourse.bass as bass
import concourse.mybir as mybir
import concourse.tile as tile
from concourse import bacc

T = 1024          # tokens per batch element (32*32)
C = 512           # channels
HEADS = 8
HC = C // HEADS   # 64
P = 128           # partitions
NT = T // P       # 8 t-tiles (also 8 s-tiles)
NCT = C // P      # 4 c-tiles
CHUNK = 512       # matmul moving-operand chunk (one fp32 PSUM bank)
NCH = T // CHUNK  # 2 chunks
NPAIR = HEADS // 2
F32 = mybir.dt.float32
BF16 = mybir.dt.bfloat16
EXP_SCALE = 1.0 / math.sqrt(HC)  # (1/sqrt(sqrt(hc)))^2 applied to q.k
VSTRIDE = HC + 1  # 65: v columns + ones column per head
VAW = HEADS * VSTRIDE + (P - VSTRIDE)  # PV lhsT 128-wide reads stay in-tile


def build_program(debug_dumps=False):
    nc = bacc.Bacc("TRN2", num_devices=8, debug=False)

    x_d = nc.dram_tensor("x", [T, C], BF16, kind="ExternalInput")
    wqkv_d = nc.dram_tensor("qkv_w", [C, 3 * C], BF16, kind="ExternalInput")
    wout_d = nc.dram_tensor("out_w", [C, C], BF16, kind="ExternalInput")
    qkb_d = nc.dram_tensor("qk_b", [2 * C], F32, kind="ExternalInput")
    ob_d = nc.dram_tensor("out_b", [C], F32, kind="ExternalInput")
    out_d = nc.dram_tensor("out", [T, C], BF16, kind="ExternalOutput")
    dbg = {}
    if debug_dumps:
        for nm, shp in [
            ("dbg_xT", [P, NCT * T]), ("dbg_q0", [P, T]), ("dbg_k0", [P, T]),
            ("dbg_exh0", [P, 2 * T]), ("dbg_va0", [P, VAW]),
            ("dbg_an0", [P, T]),
        ]:
            dbg[nm] = nc.dram_tensor(nm, shp, BF16, kind="ExternalOutput")

    with tile.TileContext(nc) as tc, ExitStack() as ctx:
        from concourse.masks import make_identity

        # ---------------- SBUF pools ----------------
        const = ctx.enter_context(tc.tile_pool(name="const", bufs=1))
        persist = ctx.enter_context(tc.tile_pool(name="persist", bufs=1))
        workp = ctx.enter_context(tc.tile_pool(name="workp", bufs=1))

        # x tiles first on the HWDGE queue: they gate the transpose pipeline
        xts = []
        for i in range(NT):
            xt_in = persist.tile([P, C], BF16, tag=f"x_in{i}", name=f"x_in{i}")
            nc.sync.dma_start(xt_in[:], x_d.ap()[i * P:(i + 1) * P, :])
            xts.append(xt_in)

        identity = const.tile([P, P], BF16, tag="ident", name="ident")
        make_identity(nc, identity[:])
        warm_rhs = const.tile([P, CHUNK], BF16, tag="warm", name="warm_rhs")
        nc.gpsimd.memset(warm_rhs[:], 0.0)

        # qkv weights: q/k columns first (they gate the qk projections)
        wq = []  # [c-tile][128, 1536] bf16
        for m in range(NCT):
            t_ = persist.tile([P, 3 * C], BF16, tag=f"wq{m}", name=f"wq{m}")
            nc.sync.dma_start(t_[:, 0:2 * C],
                             wqkv_d.ap()[m * P:(m + 1) * P, 0:2 * C])
            wq.append(t_)
        for m in range(NCT):
            nc.gpsimd.dma_start(wq[m][:, 2 * C:3 * C],
                                wqkv_d.ap()[m * P:(m + 1) * P, 2 * C:3 * C])

        # bias tiles; column m = qk_b[128m:128m+128]
        qkb_all = const.tile([P, 2 * C // P], F32, tag="qkball", name="qkb_all")
        nc.gpsimd.dma_start(
            qkb_all[:], qkb_d.ap().rearrange("(m p) -> p m", p=P)
        )
        qkb_t = [qkb_all[:, m:m + 1] for m in range(2 * C // P)]
        ob_row = const.tile([1, C], F32, tag="obrow", name="ob_row")
        nc.gpsimd.dma_start(ob_row[:], ob_d.ap().rearrange("(o c) -> o c", o=1))
        ob_bcast = const.tile([P, C], F32, tag="obb", name="ob_bcast")
        nc.gpsimd.partition_broadcast(ob_bcast[:], ob_row[:], channels=P)

        # out-proj weights: needed only in phase 3
        wo = []  # [c-tile][128, 512] bf16
        for m in range(NCT):
            t_ = persist.tile([P, C], BF16, tag=f"wo{m}", name=f"wo{m}")
            nc.gpsimd.dma_start(t_[:], wout_d.ap()[m * P:(m + 1) * P, :])
            wo.append(t_)

        # persistent activation tiles
        # xT_all[:, cc*T + t] = x^T tile cc: [c-within-tile, t]
        xT_all = persist.tile([P, NCT * T], BF16, tag="xT", name="xT_all")
        qkT = [persist.tile([P, T], BF16, tag=f"qk{m}", name=f"qk{m}")
               for m in range(NCT)]
        # kT2[m]: rows 0:64 = k^T head 2m, rows 64:128 = k^T head 2m+1
        kT2 = [persist.tile([P, T], BF16, tag=f"k2{m}", name=f"k2{m}")
               for m in range(NCT)]
        vaug = [persist.tile([P, VAW], BF16, tag=f"va{i}", name=f"va{i}")
                for i in range(NT)]
        for i in range(NT):
            nc.gpsimd.memset(vaug[i][:], 1.0)  # ones column (+padding) preset
        anorm = [persist.tile([P, T], BF16, tag=f"an{m}", name=f"an{m}")
                 for m in range(NCT)]

        # ================= prologue =================
        # HAM warm-up: real (non-transpose) matmuls on junk data while the x
        # DMA lands, so the PE clock is at 8/8 when the real work starts.
        with tc.tile_pool(name="ps_warm", bufs=1, space="PSUM") as ps_warm:
            ps_w = ps_warm.tile([P, CHUNK], F32, tag="w", name="ps_w")
            for _ in range(5):
                nc.tensor.matmul(ps_w[:], identity[:], warm_rhs[:],
                                 start=True, stop=True)

        with tc.tile_pool(name="ps_tr", bufs=2, space="PSUM") as ps_trp:
            # x PE transpose; xT_all[:, cc*T + i*128 : ...] gets tile (i, cc)
            for i in range(NT):
                ps_tr = ps_trp.tile([P, C], BF16, tag="tr", name="ps_tr")
                for cc in range(NCT):
                    nc.tensor.transpose(
                        ps_tr[:, cc * P:(cc + 1) * P],
                        xts[i][:, cc * P:(cc + 1) * P],
                        identity[:],
                    )
                # one strided copy: dest [128, cc, 128] with cc-stride T
                dst = xT_all[:].rearrange("p (cc t) -> p cc t", cc=NCT)
                nc.vector.tensor_copy(
                    dst[:, :, i * P:(i + 1) * P],
                    ps_tr[:].rearrange("p (cc q) -> p cc q", cc=NCT),
                )

        # ---------------- phase-1 emitters (woven into phase 2) ----------------
        # qk projection: per (m-tile, chunk): 4 chained MMs + bias-add.
        # m 0..3 -> q tiles (qkT), m 4..7 -> k tiles (kT2[m-4]).
        qk_cm = tc.tile_pool(name="ps_qk", bufs=2, space="PSUM", side="right")
        qk_pool = qk_cm.__enter__()

        def emit_qk(m, j):
            ps_qk = qk_pool.tile([P, CHUNK], F32, tag="qk", name="ps_qk")
            js = slice(j * CHUNK, (j + 1) * CHUNK)
            for cc in range(NCT):
                nc.tensor.matmul(
                    ps_qk[:],
                    wq[cc][:, m * P:(m + 1) * P],
                    xT_all[:, cc * T + j * CHUNK: cc * T + (j + 1) * CHUNK],
                    start=(cc == 0),
                    stop=(cc == NCT - 1),
                )
            dstt = qkT[m] if m < NCT else kT2[m - NCT]
            nc.vector.tensor_scalar_add(dstt[:, js], ps_qk[:], qkb_t[m][:])

        def emit_v(i):
            ps_v = qk_pool.tile([P, CHUNK], F32, tag="qk", name="ps_v")
            for cc in range(NCT):
                nc.tensor.matmul(
                    ps_v[:],
                    xT_all[:, cc * T + i * P: cc * T + (i + 1) * P],
                    wq[cc][:, 2 * C:3 * C],
                    start=(cc == 0),
                    stop=(cc == NCT - 1),
                )
            va3 = vaug[i][:, 0:HEADS * VSTRIDE].rearrange(
                "p (h d) -> p h d", d=VSTRIDE)
            nc.vector.tensor_copy(
                va3[:, :, 0:HC],
                ps_v[:].rearrange("p (h d) -> p h d", h=HEADS),
            )

        # prologue part 2: pair 0's operands (q tile 0, k tile 0)
        for j in range(NCH):
            emit_qk(0, j)
        for j in range(NCH):
            emit_qk(NCT, j)

        # filler iterator: remaining phase-1 work in dependency-safe order
        def phase1_fillers():
            for m in [1, NCT + 1, 2, NCT + 2, 3, NCT + 3]:
                for j in range(NCH):
                    yield ("qk", m, j)
            for i in range(NT):
                yield ("v", i)

        fillers = phase1_fillers()
        state = {"fill_done": False, "qk_cm": qk_cm, "pv_pool": None}

        def run_fillers(n):
            for _ in range(n):
                try:
                    f = next(fillers)
                except StopIteration:
                    state["fill_done"] = True
                    return
                if f[0] == "qk":
                    emit_qk(f[1], f[2])
                else:
                    emit_v(f[1])

        # ================= phase 2: attention =================
        def emit_pv_chain(h, j, ppv, exh):
            base = (h % 2) * T
            for ssi in range(NT):
                nc.tensor.matmul(
                    ppv[:],
                    vaug[ssi][:, h * VSTRIDE: h * VSTRIDE + P],
                    exh[:, ssi * 2 * T + base + j * CHUNK:
                        ssi * 2 * T + base + (j + 1) * CHUNK],
                    start=(ssi == 0),
                    stop=(ssi == NT - 1),
                )

        def emit_normalize(h, ppv0, ppv1):
            m = h // 2
            rlo = (h % 2) * HC
            dtmp = workp.tile([1, T], F32, tag="dtmp", name="dtmp")
            nc.vector.tensor_copy(dtmp[:, 0:CHUNK], ppv0[HC:HC + 1, :])
            nc.vector.tensor_copy(dtmp[:, CHUNK:T], ppv1[HC:HC + 1, :])
            recip = workp.tile([1, T], F32, tag="recip", name="recip")
            nc.vector.reciprocal_approx_fast(recip[:], dtmp[:])
            bcast = workp.tile([HC, T], F32, tag="bcast", name="bcast")
            nc.gpsimd.partition_broadcast(bcast[:], recip[:], channels=HC)
            for j, ppv in ((0, ppv0), (1, ppv1)):
                nc.vector.tensor_tensor(
                    anorm[m][rlo:rlo + HC, j * CHUNK:(j + 1) * CHUNK],
                    ppv[0:HC, :],
                    bcast[:, j * CHUNK:(j + 1) * CHUNK],
                    op=mybir.AluOpType.mult,
                )

        def make_pv_steps(p, exh):
            """Six lazy closures: (j0 chain, j1 chain, normalize) x 2 heads."""
            steps = []
            for hh in range(2):
                h = 2 * p + hh
                box = {}

                def s_j0(h=h, box=box):
                    if state["pv_pool"] is None:
                        # opened lazily AFTER the qk pool closes (PSUM budget)
                        state["pv_cm"] = tc.tile_pool(
                            name="ps_pv", bufs=2, space="PSUM", side="right")
                        state["pv_pool"] = state["pv_cm"].__enter__()
                    box["p0"] = state["pv_pool"].tile(
                        [P, CHUNK], F32, tag="pv", name="ppv0")
                    emit_pv_chain(h, 0, box["p0"], exh)

                def s_j1(h=h, box=box):
                    box["p1"] = state["pv_pool"].tile(
                        [P, CHUNK], F32, tag="pv", name="ppv1")
                    emit_pv_chain(h, 1, box["p1"], exh)

                def s_norm(h=h, box=box):
                    emit_normalize(h, box["p0"], box["p1"])

                steps += [s_j0, s_j1, s_norm]
            return steps

        exh_pool = ctx.enter_context(tc.tile_pool(name="exh", bufs=2))
        st_cm = tc.tile_pool(name="ps_st", bufs=3, space="PSUM")
        st_pool = st_cm.__enter__()

        pending = []  # queue of PV/normalize closures for the previous pair
        exhs = []
        for p in range(NPAIR):
            exh = exh_pool.tile([P, NT * 2 * T], BF16, tag="exh", name="exh")
            exhs.append(exh)
            if debug_dumps and p == 1:
                nc.sync.dma_start(dbg["dbg_exh0"].ap(), exhs[0][:, 0:2 * T])
            if p > 0:
                pending.extend(make_pv_steps(p - 1, exhs[p - 1]))
            for ssi in range(NT):
                sta = st_pool.tile([P, T], F32, tag="st", name="sta")
                stb = st_pool.tile([P, T], F32, tag="st", name="stb")
                for j in range(NCH):
                    js = slice(j * CHUNK, (j + 1) * CHUNK)
                    nc.tensor.matmul(
                        sta[:, js],
                        kT2[p][0:HC, ssi * P:(ssi + 1) * P],
                        qkT[p][0:HC, js],
                        start=True, stop=True,
                    )
                    nc.tensor.matmul(
                        stb[:, js],
                        kT2[p][HC:P, ssi * P:(ssi + 1) * P],
                        qkT[p][HC:P, js],
                        start=True, stop=True,
                    )
                nc.scalar.activation(
                    exh[:, ssi * 2 * T: ssi * 2 * T + T],
                    sta[:],
                    mybir.ActivationFunctionType.Exp,
                    scale=EXP_SCALE,
                )
                nc.scalar.activation(
                    exh[:, ssi * 2 * T + T: (ssi + 1) * 2 * T],
                    stb[:],
                    mybir.ActivationFunctionType.Exp,
                    scale=EXP_SCALE,
                )
                # PE-gap fillers for this slot
                if not state["fill_done"]:
                    run_fillers(2)
                else:
                    if state["qk_cm"] is not None:
                        state["qk_cm"].__exit__(None, None, None)
                        state["qk_cm"] = None
                    if pending:
                        pending.pop(0)()
            # end of pair: drain anything left for pair p-1
            while pending:
                pending.pop(0)()
        if state["qk_cm"] is not None:
            state["qk_cm"].__exit__(None, None, None)
            state["qk_cm"] = None
        # last pair's PV + normalize (tail)
        for step in make_pv_steps(NPAIR - 1, exhs[-1]):
            step()
        st_cm.__exit__(None, None, None)
        if state["pv_pool"] is not None:
            state["pv_cm"].__exit__(None, None, None)
            state["pv_pool"] = None

        if debug_dumps:
            nc.sync.dma_start(dbg["dbg_xT"].ap(), xT_all[:])
            nc.sync.dma_start(dbg["dbg_q0"].ap(), qkT[0][:])
            nc.sync.dma_start(dbg["dbg_k0"].ap(), kT2[0][:])
            nc.sync.dma_start(dbg["dbg_va0"].ap(), vaug[0][:])
            nc.sync.dma_start(dbg["dbg_an0"].ap(), anorm[0][:])

        # ================= phase 3: out projection =================
        with tc.tile_pool(name="ps_o", bufs=2, space="PSUM") as ps_op:
            for i in range(NT):
                ps_o = ps_op.tile([P, C], F32, tag="o", name="ps_o")
                for cc in range(NCT):
                    nc.tensor.matmul(
                        ps_o[:],
                        anorm[cc][:, i * P:(i + 1) * P],
                        wo[cc][:],
                        start=(cc == 0),
                        stop=(cc == NCT - 1),
                    )
                osb = workp.tile([P, C], BF16, tag=f"osb{i}", name=f"osb{i}")
                nc.vector.tensor_tensor(
                    osb[:], ps_o[:], ob_bcast[:], op=mybir.AluOpType.add)
                nc.sync.dma_start(out_d.ap()[i * P:(i + 1) * P, :], osb[:])

    nc.compile()
    return nc


_CACHED_NC = None


def _get_nc():
    global _CACHED_NC
    if _CACHED_NC is None:
        _CACHED_NC = build_program(
            debug_dumps=bool(int(os.environ.get("KERNEL_DEBUG", "0"))))
    return _CACHED_NC


def _prep_inputs(x, qkv_w, qkv_b, out_w, out_b):
    import ml_dtypes

    x = np.asarray(x)
    B = x.shape[0]
    x2 = x.reshape(B, T, C).astype(ml_dtypes.bfloat16)
    wq2 = np.asarray(qkv_w).reshape(C, 3 * C).astype(ml_dtypes.bfloat16)
    wo2 = np.asarray(out_w).reshape(C, C).astype(ml_dtypes.bfloat16)
    qkv_b = np.asarray(qkv_b).astype(np.float32)
    out_b = np.asarray(out_b).astype(np.float32)
    # fold the v-bias through the output projection (exact: A_norm += b_v
    # shifts out by b_v @ W_out since softmax rows sum to 1).
    b_v = qkv_b[2 * C:3 * C]
    ob_eff = (
        out_b.astype(np.float64)
        + b_v.astype(np.float64) @ wo2.astype(np.float64)
    ).astype(np.float32)
    qkb = np.ascontiguousarray(qkv_b[0:2 * C])
    return x2, wq2, wo2, qkb, ob_eff


def kernel(x, qkv_w, qkv_b, out_w, out_b):
    """Full inputs in, full output out.  Shards batch across 8 NeuronCores."""
    from concourse.bass_utils import run_bass_kernel_spmd

    x = np.asarray(x)
    B, H, W, Cc = x.shape
    assert (B, H, W, Cc) == (8, 32, 32, C)
    x2, wq2, wo2, qkb, ob_eff = _prep_inputs(x, qkv_w, qkv_b, out_w, out_b)

    nc = _get_nc()
    in_maps = [
        {
            "x": np.ascontiguousarray(x2[b]),
            "qkv_w": np.ascontiguousarray(wq2),
            "out_w": np.ascontiguousarray(wo2),
            "qk_b": qkb,
            "out_b": ob_eff,
        }
        for b in range(B)
    ]
    trace = bool(int(os.environ.get("KERNEL_TRACE", "0")))
    res = run_bass_kernel_spmd(nc, in_maps, core_ids=list(range(B)), trace=trace)
    if trace and res.exec_time_ns is not None:
        print(f"HW exec time: {res.exec_time_ns} ns")
    kernel.last_results = res
    out = np.stack(
        [np.asarray(res.results[b]["out"]).astype(np.float32) for b in range(B)],
        axis=0,
    )
    return out.reshape(B, H, W, Cc)


kernel.last_results = None


# revision 20
# speedup vs baseline: 1.4017x; 1.1512x over previous
"""Trainium2 Bass kernel for nn_AttentionBlock (B=8, H=W=32, C=512, 8 heads).

Strategy: data-parallel over batch -- each of the 8 NeuronCores processes one
batch element end-to-end (no collectives).  Per core:

  x [T=1024, C=512] -> qkv -> per-head attention (T x T softmax) -> out proj.

v2 design (ACT-bound pipeline):
  * all matmul data is bf16 (host pre-casts); PSUM accumulation stays fp32.
  * S^T = k^T q is computed with K=64 ROW-TILED matmul pairs: head 2p on PE
    row-groups 0-1 (SBUF partitions 0:64), head 2p+1 on row-groups 2-3
    (partitions 64:128).  The two matmuls run concurrently in the array, so
    a head-pair s-tile costs ~2x512 columns instead of 4x512.
  * softmax exp runs on ScalarE (the only exp engine) in N=1024 calls and is
    the phase-2 bottleneck (~73us); everything else (QKV projection, PV,
    out-projection) is woven into the PE gaps between exp calls.
  * denominators come free from a ones-column appended to V (PV row 64);
    normalization = DVE multiply by a GPSIMD-broadcast reciprocal.
  * out-projection uses anorm tiles as lhsT so the output lands directly in
    [t, c] layout -- no output transpose.  Output is bf16; host casts back.
  * no max-subtraction: logits are ~N(0,1) by construction (1/8 scale is
    folded into the ScalarE exp activation).
"""

import math
import os
from contextlib import ExitStack

import numpy as np

import concourse.bass as bass
import concourse.mybir as mybir
import concourse.tile as tile
from concourse import bacc

T = 1024          # tokens per batch element (32*32)
C = 512           # channels
HEADS = 8
HC = C // HEADS   # 64
P = 128           # partitions
NT = T // P       # 8 t-tiles (also 8 s-tiles)
NCT = C // P      # 4 c-tiles
CHUNK = 512       # matmul moving-operand chunk (one fp32 PSUM bank)
NCH = T // CHUNK  # 2 chunks
NPAIR = HEADS // 2
F32 = mybir.dt.float32
BF16 = mybir.dt.bfloat16
EXP_SCALE = 1.0 / math.sqrt(HC)  # (1/sqrt(sqrt(hc)))^2 applied to q.k
VSTRIDE = HC + 1  # 65: v columns + ones column per head
VAW = HEADS * VSTRIDE + (P - VSTRIDE)  # PV lhsT 128-wide reads stay in-tile


def build_program(debug_dumps=False):
    nc = bacc.Bacc("TRN2", num_devices=8, debug=False)

    x_d = nc.dram_tensor("x", [T, C], BF16, kind="ExternalInput")
    wqkv_d = nc.dram_tensor("qkv_w", [C, 3 * C], BF16, kind="ExternalInput")
    wout_d = nc.dram_tensor("out_w", [C, C], BF16, kind="ExternalInput")
    qkb_d = nc.dram_tensor("qk_b", [2 * C], F32, kind="ExternalInput")
    ob_d = nc.dram_tensor("out_b", [C], F32, kind="ExternalInput")
    out_d = nc.dram_tensor("out", [T, C], BF16, kind="ExternalOutput")
    dbg = {}
    if debug_dumps:
        for nm, shp in [
            ("dbg_xT", [P, NCT * T]), ("dbg_q0", [P, T]), ("dbg_k0", [P, T]),
            ("dbg_exh0", [P, 2 * T]), ("dbg_va0", [P, VAW]),
            ("dbg_an0", [P, T]),
        ]:
            dbg[nm] = nc.dram_tensor(nm, shp, BF16, kind="ExternalOutput")

    with tile.TileContext(nc) as tc, ExitStack() as ctx:
        from concourse.masks import make_identity

        # ---------------- SBUF pools ----------------
        const = ctx.enter_context(tc.tile_pool(name="const", bufs=1))
        persist = ctx.enter_context(tc.tile_pool(name="persist", bufs=1))
        workp = ctx.enter_context(tc.tile_pool(name="workp", bufs=1))

        # x tiles first on the HWDGE queues (split across sync/scalar so the
        # descriptor-issue cost doesn't serialize): they gate the transposes
        xts = []
        for i in range(NT):
            xt_in = persist.tile([P, C], BF16, tag=f"x_in{i}", name=f"x_in{i}")
            eng = nc.sync if i % 2 == 0 else nc.scalar
            eng.dma_start(xt_in[:], x_d.ap()[i * P:(i + 1) * P, :])
            xts.append(xt_in)

        identity = const.tile([P, P], BF16, tag="ident", name="ident")
        make_identity(nc, identity[:])
        warm_rhs = const.tile([P, CHUNK], BF16, tag="warm", name="warm_rhs")
        nc.gpsimd.memset(warm_rhs[:], 0.0)

        # qkv weights: q/k columns first (they gate the qk projections)
        wq = []  # [c-tile][128, 1536] bf16
        for m in range(NCT):
            t_ = persist.tile([P, 3 * C], BF16, tag=f"wq{m}", name=f"wq{m}")
            eng = nc.sync if m % 2 == 0 else nc.scalar
            eng.dma_start(t_[:, 0:2 * C],
                          wqkv_d.ap()[m * P:(m + 1) * P, 0:2 * C])
            wq.append(t_)
        for m in range(NCT):
            nc.gpsimd.dma_start(wq[m][:, 2 * C:3 * C],
                                wqkv_d.ap()[m * P:(m + 1) * P, 2 * C:3 * C])

        # bias tiles; column m = qk_b[128m:128m+128]
        qkb_all = const.tile([P, 2 * C // P], F32, tag="qkball", name="qkb_all")
        nc.gpsimd.dma_start(
            qkb_all[:], qkb_d.ap().rearrange("(m p) -> p m", p=P)
        )
        qkb_t = [qkb_all[:, m:m + 1] for m in range(2 * C // P)]
        ob_row = const.tile([1, C], F32, tag="obrow", name="ob_row")
        nc.gpsimd.dma_start(ob_row[:], ob_d.ap().rearrange("(o c) -> o c", o=1))
        ob_bcast = const.tile([P, C], F32, tag="obb", name="ob_bcast")
        nc.gpsimd.partition_broadcast(ob_bcast[:], ob_row[:], channels=P)

        # out-proj weights: needed only in phase 3
        wo = []  # [c-tile][128, 512] bf16
        for m in range(NCT):
            t_ = persist.tile([P, C], BF16, tag=f"wo{m}", name=f"wo{m}")
            nc.gpsimd.dma_start(t_[:], wout_d.ap()[m * P:(m + 1) * P, :])
            wo.append(t_)

        # persistent activation tiles
        # xT_all[:, cc*T + t] = x^T tile cc: [c-within-tile, t]
        xT_all = persist.tile([P, NCT * T], BF16, tag="xT", name="xT_all")
        qkT = [persist.tile([P, T], BF16, tag=f"qk{m}", name=f"qk{m}")
               for m in range(NCT)]
        # kT2[m]: rows 0:64 = k^T head 2m, rows 64:128 = k^T head 2m+1
        kT2 = [persist.tile([P, T], BF16, tag=f"k2{m}", name=f"k2{m}")
               for m in range(NCT)]
        vaug = [persist.tile([P, VAW], BF16, tag=f"va{i}", name=f"va{i}")
                for i in range(NT)]
        for i in range(NT):
            nc.gpsimd.memset(vaug[i][:], 1.0)  # ones column (+padding) preset
        anorm = [persist.tile([P, T], BF16, tag=f"an{m}", name=f"an{m}")
                 for m in range(NCT)]

        # ================= prologue =================
        # HAM warm-up: real (non-transpose) matmuls on junk data while the x
        # DMA lands, so the PE clock is at 8/8 when the real work starts.
        with tc.tile_pool(name="ps_warm", bufs=1, space="PSUM") as ps_warm:
            ps_w = ps_warm.tile([P, CHUNK], F32, tag="w", name="ps_w")
            for _ in range(5):
                nc.tensor.matmul(ps_w[:], identity[:], warm_rhs[:],
                                 start=True, stop=True)

        # exp ACT-table preload: a tiny dummy exp during the DMA wait pays the
        # ~2.7us one-time table-load cost before the real pipeline needs it.
        scratch16 = workp.tile([1, 16], F32, tag="scr16", name="scratch16")
        nc.scalar.activation(
            scratch16[:], warm_rhs[0:1, 0:16],
            mybir.ActivationFunctionType.Exp, scale=1.0)

        # st pool first on the left stack (outlives the qk pool)
        st_cm = tc.tile_pool(name="ps_st", bufs=2, space="PSUM")
        st_pool = st_cm.__enter__()
        qk_cm = tc.tile_pool(name="ps_qk", bufs=2, space="PSUM")
        qk_pool = qk_cm.__enter__()

        tr_cm = tc.tile_pool(name="ps_tr", bufs=2, space="PSUM", side="right")
        tr_pool = tr_cm.__enter__()

        def emit_transpose(i):
            # x PE transpose; xT_all[:, cc*T + i*128 : ...] gets tile (i, cc)
            ps_tr = tr_pool.tile([P, C], BF16, tag="tr", name="ps_tr")
            for cc in range(NCT):
                nc.tensor.transpose(
                    ps_tr[:, cc * P:(cc + 1) * P],
                    xts[i][:, cc * P:(cc + 1) * P],
                    identity[:],
                )
            # one strided copy: dest [128, cc, 128] with cc-stride T
            dst = xT_all[:].rearrange("p (cc t) -> p cc t", cc=NCT)
            nc.vector.tensor_copy(
                dst[:, :, i * P:(i + 1) * P],
                ps_tr[:].rearrange("p (cc q) -> p cc q", cc=NCT),
            )

        def emit_qk(m, j):
            ps_qk = qk_pool.tile([P, CHUNK], F32, tag="qk", name="ps_qk")
            js = slice(j * CHUNK, (j + 1) * CHUNK)
            for cc in range(NCT):
                nc.tensor.matmul(
                    ps_qk[:],
                    wq[cc][:, m * P:(m + 1) * P],
                    xT_all[:, cc * T + j * CHUNK: cc * T + (j + 1) * CHUNK],
                    start=(cc == 0),
                    stop=(cc == NCT - 1),
                )
            dstt = qkT[m] if m < NCT else kT2[m - NCT]
            nc.vector.tensor_scalar_add(dstt[:, js], ps_qk[:], qkb_t[m][:])

        def emit_v(i):
            ps_v = qk_pool.tile([P, CHUNK], F32, tag="qk", name="ps_v")
            for cc in range(NCT):
                nc.tensor.matmul(
                    ps_v[:],
                    xT_all[:, cc * T + i * P: cc * T + (i + 1) * P],
                    wq[cc][:, 2 * C:3 * C],
                    start=(cc == 0),
                    stop=(cc == NCT - 1),
                )
            va3 = vaug[i][:, 0:HEADS * VSTRIDE].rearrange(
                "p (h d) -> p h d", d=VSTRIDE)
            nc.vector.tensor_copy(
                va3[:, :, 0:HC],
                ps_v[:].rearrange("p (h d) -> p h d", h=HEADS),
            )

        # prologue part 2: transposes interleaved with pair 0's q/k tiles so
        # the first S^T slot is reachable as early as possible.  Chunk j of
        # qkT[0]/kT2[0] needs x tiles 4j..4j+3 transposed.
        for i in range(NT // 2):
            emit_transpose(i)
        emit_qk(0, 0)
        emit_qk(NCT, 0)
        for i in range(NT // 2, NT):
            emit_transpose(i)
        emit_qk(0, 1)
        emit_qk(NCT, 1)
        tr_cm.__exit__(None, None, None)

        # filler iterator: remaining phase-1 work in dependency-safe order
        def phase1_fillers():
            for m in [1, NCT + 1, 2, NCT + 2, 3, NCT + 3]:
                for j in range(NCH):
                    yield ("qk", m, j)
            for i in range(NT):
                yield ("v", i)

        fillers = phase1_fillers()
        state = {"fill_done": False, "qk_cm": qk_cm, "pv_pool": None}

        def run_fillers(n):
            for _ in range(n):
                try:
                    f = next(fillers)
                except StopIteration:
                    state["fill_done"] = True
                    return
                if f[0] == "qk":
                    emit_qk(f[1], f[2])
                else:
                    emit_v(f[1])

        # ================= phase 2: attention =================
        def emit_pv_chain(h, j, ppv, exh):
            base = (h % 2) * T
            for ssi in range(NT):
                nc.tensor.matmul(
                    ppv[:],
                    vaug[ssi][:, h * VSTRIDE: h * VSTRIDE + P],
                    exh[:, ssi * 2 * T + base + j * CHUNK:
                        ssi * 2 * T + base + (j + 1) * CHUNK],
                    start=(ssi == 0),
                    stop=(ssi == NT - 1),
                )

        def emit_normalize(h, ppv0, ppv1):
            m = h // 2
            rlo = (h % 2) * HC
            dtmp = workp.tile([1, T], F32, tag="dtmp", name="dtmp")
            nc.vector.tensor_copy(dtmp[:, 0:CHUNK], ppv0[HC:HC + 1, :])
            nc.vector.tensor_copy(dtmp[:, CHUNK:T], ppv1[HC:HC + 1, :])
            recip = workp.tile([1, T], F32, tag="recip", name="recip")
            nc.vector.reciprocal_approx_fast(recip[:], dtmp[:])
            bcast = workp.tile([HC, T], F32, tag="bcast", name="bcast")
            nc.gpsimd.partition_broadcast(bcast[:], recip[:], channels=HC)
            for j, ppv in ((0, ppv0), (1, ppv1)):
                nc.vector.tensor_tensor(
                    anorm[m][rlo:rlo + HC, j * CHUNK:(j + 1) * CHUNK],
                    ppv[0:HC, :],
                    bcast[:, j * CHUNK:(j + 1) * CHUNK],
                    op=mybir.AluOpType.mult,
                )

        def make_pv_steps(p, exh):
            """Six lazy closures: (j0 chain, j1 chain, normalize) x 2 heads.
            Each chain gets its own dedicated PSUM bank (tags pv0..pv3) so a
            chain never WAR-stalls behind the other head's normalize."""
            steps = []
            for hh in range(2):
                h = 2 * p + hh
                box = {}

                def s_j0(h=h, hh=hh, box=box):
                    if state["pv_pool"] is None:
                        # opened lazily AFTER the qk pool closes (PSUM budget)
                        state["pv_cm"] = tc.tile_pool(
                            name="ps_pv", bufs=1, space="PSUM", side="right")
                        state["pv_pool"] = state["pv_cm"].__enter__()
                    box["p0"] = state["pv_pool"].tile(
                        [P, CHUNK], F32, tag=f"pv{2 * hh}", name="ppv0")
                    emit_pv_chain(h, 0, box["p0"], exh)

                def s_j1(h=h, hh=hh, box=box):
                    box["p1"] = state["pv_pool"].tile(
                        [P, CHUNK], F32, tag=f"pv{2 * hh + 1}", name="ppv1")
                    emit_pv_chain(h, 1, box["p1"], exh)

                def s_norm(h=h, box=box):
                    emit_normalize(h, box["p0"], box["p1"])

                steps += [s_j0, s_j1, s_norm]
            return steps

        exh_pool = ctx.enter_context(tc.tile_pool(name="exh", bufs=2))

        # slot list across all pairs; S^T matmuls are emitted ONE SLOT AHEAD
        # of their exp so the ACT-critical feed sits in front of any payload
        # stalls in the PE FIFO.
        slots = [(p, ssi) for p in range(NPAIR) for ssi in range(NT)]
        exhs = []
        st_q = []

        def ensure_exh(p):
            while len(exhs) <= p:
                exhs.append(exh_pool.tile(
                    [P, NT * 2 * T], BF16, tag="exh", name="exh"))

        def emit_st(p, ssi):
            ensure_exh(p)
            sta = st_pool.tile([P, T], F32, tag="st", name="sta")
            stb = st_pool.tile([P, T], F32, tag="st", name="stb")
            for j in range(NCH):
                js = slice(j * CHUNK, (j + 1) * CHUNK)
                nc.tensor.matmul(
                    sta[:, js],
                    kT2[p][0:HC, ssi * P:(ssi + 1) * P],
                    qkT[p][0:HC, js],
                    start=True, stop=True,
                )
                nc.tensor.matmul(
                    stb[:, js],
                    kT2[p][HC:P, ssi * P:(ssi + 1) * P],
                    qkT[p][HC:P, js],
                    start=True, stop=True,
                )
            st_q.append((sta, stb))

        pending = []  # queue of PV/normalize closures for the previous pair
        emit_st(*slots[0])
        for g, (p, ssi) in enumerate(slots):
            exh = exhs[p]
            sta, stb = st_q.pop(0)
            nc.scalar.activation(
                exh[:, ssi * 2 * T: ssi * 2 * T + T],
                sta[:],
                mybir.ActivationFunctionType.Exp,
                scale=EXP_SCALE,
            )
            nc.scalar.activation(
                exh[:, ssi * 2 * T + T: (ssi + 1) * 2 * T],
                stb[:],
                mybir.ActivationFunctionType.Exp,
                scale=EXP_SCALE,
            )
            # next slot's S^T goes in front of this slot's payload work
            if g + 1 < len(slots):
                emit_st(*slots[g + 1])
            if ssi == 0 and p > 0:
                pending.extend(make_pv_steps(p - 1, exhs[p - 1]))
                if debug_dumps and p == 1:
                    nc.sync.dma_start(
                        dbg["dbg_exh0"].ap(), exhs[0][:, 0:2 * T])
            # PE-gap payloads for this slot
            if not state["fill_done"]:
                run_fillers(2)
            else:
                if state["qk_cm"] is not None:
                    state["qk_cm"].__exit__(None, None, None)
                    state["qk_cm"] = None
                if pending:
                    pending.pop(0)()
        if state["qk_cm"] is not None:
            state["qk_cm"].__exit__(None, None, None)
            state["qk_cm"] = None
        # tail: whatever remains of pair 2's PV, then pair 3's PV + normalize
        pending.extend(make_pv_steps(NPAIR - 1, exhs[-1]))
        while pending:
            pending.pop(0)()
        st_cm.__exit__(None, None, None)
        if state["pv_pool"] is not None:
            state["pv_cm"].__exit__(None, None, None)
            state["pv_pool"] = None

        if debug_dumps:
            nc.sync.dma_start(dbg["dbg_xT"].ap(), xT_all[:])
            nc.sync.dma_start(dbg["dbg_q0"].ap(), qkT[0][:])
            nc.sync.dma_start(dbg["dbg_k0"].ap(), kT2[0][:])
            nc.sync.dma_start(dbg["dbg_va0"].ap(), vaug[0][:])
            nc.sync.dma_start(dbg["dbg_an0"].ap(), anorm[0][:])

        # ================= phase 3: out projection =================
        with tc.tile_pool(name="ps_o", bufs=2, space="PSUM") as ps_op:
            for i in range(NT):
                ps_o = ps_op.tile([P, C], F32, tag="o", name="ps_o")
                for cc in range(NCT):
                    nc.tensor.matmul(
                        ps_o[:],
                        anorm[cc][:, i * P:(i + 1) * P],
                        wo[cc][:],
                        start=(cc == 0),
                        stop=(cc == NCT - 1),
                    )
                osb = workp.tile([P, C], BF16, tag=f"osb{i}", name=f"osb{i}")
                nc.vector.tensor_tensor(
                    osb[:], ps_o[:], ob_bcast[:], op=mybir.AluOpType.add)
                nc.sync.dma_start(out_d.ap()[i * P:(i + 1) * P, :], osb[:])

    nc.compile()
    return nc


_CACHED_NC = None


def _get_nc():
    global _CACHED_NC
    if _CACHED_NC is None:
        _CACHED_NC = build_program(
            debug_dumps=bool(int(os.environ.get("KERNEL_DEBUG", "0"))))
    return _CACHED_NC


def _prep_inputs(x, qkv_w, qkv_b, out_w, out_b):
    import ml_dtypes

    x = np.asarray(x)
    B = x.shape[0]
    x2 = x.reshape(B, T, C).astype(ml_dtypes.bfloat16)
    wq2 = np.asarray(qkv_w).reshape(C, 3 * C).astype(ml_dtypes.bfloat16)
    wo2 = np.asarray(out_w).reshape(C, C).astype(ml_dtypes.bfloat16)
    qkv_b = np.asarray(qkv_b).astype(np.float32)
    out_b = np.asarray(out_b).astype(np.float32)
    # fold the v-bias through the output projection (exact: A_norm += b_v
    # shifts out by b_v @ W_out since softmax rows sum to 1).
    b_v = qkv_b[2 * C:3 * C]
    ob_eff = (
        out_b.astype(np.float64)
        + b_v.astype(np.float64) @ wo2.astype(np.float64)
    ).astype(np.float32)
    qkb = np.ascontiguousarray(qkv_b[0:2 * C])
    return x2, wq2, wo2, qkb, ob_eff


def kernel(x, qkv_w, qkv_b, out_w, out_b):
    """Full inputs in, full output out.  Shards batch across 8 NeuronCores."""
    from concourse.bass_utils import run_bass_kernel_spmd

    x = np.asarray(x)
    B, H, W, Cc = x.shape
    assert (B, H, W, Cc) == (8, 32, 32, C)
    x2, wq2, wo2, qkb, ob_eff = _prep_inputs(x, qkv_w, qkv_b, out_w, out_b)

    nc = _get_nc()
    in_maps = [
        {
            "x": np.ascontiguousarray(x2[b]),
            "qkv_w": np.ascontiguousarray(wq2),
            "out_w": np.ascontiguousarray(wo2),
            "qk_b": qkb,
            "out_b": ob_eff,
        }
        for b in range(B)
    ]
    trace = bool(int(os.environ.get("KERNEL_TRACE", "0")))
    res = run_bass_kernel_spmd(nc, in_maps, core_ids=list(range(B)), trace=trace)
    if trace and res.exec_time_ns is not None:
        print(f"HW exec time: {res.exec_time_ns} ns")
    kernel.last_results = res
    out = np.stack(
        [np.asarray(res.results[b]["out"]).astype(np.float32) for b in range(B)],
        axis=0,
    )
    return out.reshape(B, H, W, Cc)


kernel.last_results = None


# revision 25
# speedup vs baseline: 1.4119x; 1.0073x over previous
"""Trainium2 Bass kernel for nn_AttentionBlock (B=8, H=W=32, C=512, 8 heads).

Strategy: data-parallel over batch -- each of the 8 NeuronCores processes one
batch element end-to-end (no collectives).  Per core:

  x [T=1024, C=512] -> qkv -> per-head attention (T x T softmax) -> out proj.

v2 design (ACT-bound pipeline):
  * all matmul data is bf16 (host pre-casts); PSUM accumulation stays fp32.
  * S^T = k^T q is computed with K=64 ROW-TILED matmul pairs: head 2p on PE
    row-groups 0-1 (SBUF partitions 0:64), head 2p+1 on row-groups 2-3
    (partitions 64:128).  The two matmuls run concurrently in the array, so
    a head-pair s-tile costs ~2x512 columns instead of 4x512.
  * softmax exp runs on ScalarE (the only exp engine) in N=1024 calls and is
    the phase-2 bottleneck (~73us); everything else (QKV projection, PV,
    out-projection) is woven into the PE gaps between exp calls.
  * denominators come free from a ones-column appended to V (PV row 64);
    normalization = DVE multiply by a GPSIMD-broadcast reciprocal.
  * out-projection uses anorm tiles as lhsT so the output lands directly in
    [t, c] layout -- no output transpose.  Output is bf16; host casts back.
  * no max-subtraction: logits are ~N(0,1) by construction (1/8 scale is
    folded into the ScalarE exp activation).
"""

import math
import os
from contextlib import ExitStack

import numpy as np

import concourse.bass as bass
import concourse.mybir as mybir
import concourse.tile as tile
from concourse import bacc

T = 1024          # tokens per batch element (32*32)
C = 512           # channels
HEADS = 8
HC = C // HEADS   # 64
P = 128           # partitions
NT = T // P       # 8 t-tiles (also 8 s-tiles)
NCT = C // P      # 4 c-tiles
CHUNK = 512       # matmul moving-operand chunk (one fp32 PSUM bank)
NCH = T // CHUNK  # 2 chunks
NPAIR = HEADS // 2
F32 = mybir.dt.float32
BF16 = mybir.dt.bfloat16
EXP_SCALE = 1.0 / math.sqrt(HC)  # (1/sqrt(sqrt(hc)))^2 applied to q.k
VSTRIDE = HC + 1  # 65: v columns + ones column per head
VAW = HEADS * VSTRIDE + (P - VSTRIDE)  # PV lhsT 128-wide reads stay in-tile


def build_program(debug_dumps=False):
    nc = bacc.Bacc("TRN2", num_devices=8, debug=False)

    x_d = nc.dram_tensor("x", [T, C], BF16, kind="ExternalInput")
    wqkv_d = nc.dram_tensor("qkv_w", [C, 3 * C], BF16, kind="ExternalInput")
    wout_d = nc.dram_tensor("out_w", [C, C], BF16, kind="ExternalInput")
    qkb_d = nc.dram_tensor("qk_b", [2 * C], F32, kind="ExternalInput")
    ob_d = nc.dram_tensor("out_b", [C], F32, kind="ExternalInput")
    out_d = nc.dram_tensor("out", [T, C], BF16, kind="ExternalOutput")
    dbg = {}
    if debug_dumps:
        for nm, shp in [
            ("dbg_xT", [P, NCT * T]), ("dbg_q0", [P, T]), ("dbg_k0", [P, T]),
            ("dbg_exh0", [P, 2 * T]), ("dbg_va0", [P, VAW]),
            ("dbg_an0", [P, T]),
        ]:
            dbg[nm] = nc.dram_tensor(nm, shp, BF16, kind="ExternalOutput")

    with tile.TileContext(nc) as tc, ExitStack() as ctx:
        from concourse.masks import make_identity

        # ---------------- SBUF pools ----------------
        const = ctx.enter_context(tc.tile_pool(name="const", bufs=1))
        persist = ctx.enter_context(tc.tile_pool(name="persist", bufs=1))
        workp = ctx.enter_context(tc.tile_pool(name="workp", bufs=1))

        # x first, as TWO batched DMAs (one per HWDGE queue): per-DMA issue
        # cost is ~0.6us, so batching beats per-tile transfers.  Layout:
        # x_in[:, i*C + c] = x[i*128 + p, c] (t-tile-major).
        x_in = persist.tile([P, NT * C], BF16, tag="x_in", name="x_in")
        xr = x_d.ap().rearrange("(i p) c -> p i c", p=P)  # [128, 8, 512]
        xv = x_in[:].rearrange("p (i c) -> p i c", i=NT)
        nc.sync.dma_start(xv[:, 0:NT // 2, :], xr[:, 0:NT // 2, :])
        nc.scalar.dma_start(xv[:, NT // 2:NT, :], xr[:, NT // 2:NT, :])

        identity = const.tile([P, P], BF16, tag="ident", name="ident")
        make_identity(nc, identity[:])
        warm_rhs = const.tile([P, CHUNK], BF16, tag="warm", name="warm_rhs")
        nc.gpsimd.memset(warm_rhs[:], 0.0)

        # qkv weights: one whole-tile DMA each, alternating HWDGE queues
        wq = []  # [c-tile][128, 1536] bf16
        for m in range(NCT):
            t_ = persist.tile([P, 3 * C], BF16, tag=f"wq{m}", name=f"wq{m}")
            eng = nc.sync if m % 2 == 0 else nc.scalar
            eng.dma_start(t_[:], wqkv_d.ap()[m * P:(m + 1) * P, :])
            wq.append(t_)

        # bias tiles; column m = qk_b[128m:128m+128]
        qkb_all = const.tile([P, 2 * C // P], F32, tag="qkball", name="qkb_all")
        nc.gpsimd.dma_start(
            qkb_all[:], qkb_d.ap().rearrange("(m p) -> p m", p=P)
        )
        qkb_t = [qkb_all[:, m:m + 1] for m in range(2 * C // P)]
        ob_row = const.tile([1, C], F32, tag="obrow", name="ob_row")
        nc.gpsimd.dma_start(ob_row[:], ob_d.ap().rearrange("(o c) -> o c", o=1))
        ob_bcast = const.tile([P, C], F32, tag="obb", name="ob_bcast")
        nc.gpsimd.partition_broadcast(ob_bcast[:], ob_row[:], channels=P)

        # out-proj weights: single batched DMA, needed only in phase 3
        wo_all = persist.tile([P, NCT * C], BF16, tag="wo", name="wo_all")
        nc.gpsimd.dma_start(
            wo_all[:].rearrange("p (m c) -> p m c", m=NCT),
            wout_d.ap().rearrange("(m p) c -> p m c", p=P),
        )
        wo = [wo_all[:, m * C:(m + 1) * C] for m in range(NCT)]

        # persistent activation tiles
        # xT_all[:, cc*T + t] = x^T tile cc: [c-within-tile, t]
        xT_all = persist.tile([P, NCT * T], BF16, tag="xT", name="xT_all")
        qkT = [persist.tile([P, T], BF16, tag=f"qk{m}", name=f"qk{m}")
               for m in range(NCT)]
        # kT2[m]: rows 0:64 = k^T head 2m, rows 64:128 = k^T head 2m+1
        kT2 = [persist.tile([P, T], BF16, tag=f"k2{m}", name=f"k2{m}")
               for m in range(NCT)]
        vaug = [persist.tile([P, VAW], BF16, tag=f"va{i}", name=f"va{i}")
                for i in range(NT)]
        for i in range(NT):
            nc.gpsimd.memset(vaug[i][:], 1.0)  # ones column (+padding) preset
        anorm = [persist.tile([P, T], BF16, tag=f"an{m}", name=f"an{m}")
                 for m in range(NCT)]

        # ================= prologue =================
        # HAM warm-up: real (non-transpose) matmuls on junk data while the x
        # DMA lands, so the PE clock is at 8/8 when the real work starts.
        with tc.tile_pool(name="ps_warm", bufs=1, space="PSUM") as ps_warm:
            ps_w = ps_warm.tile([P, CHUNK], F32, tag="w", name="ps_w")
            for _ in range(5):
                nc.tensor.matmul(ps_w[:], identity[:], warm_rhs[:],
                                 start=True, stop=True)

        # exp ACT-table preload: a tiny dummy exp during the DMA wait pays the
        # ~2.7us one-time table-load cost before the real pipeline needs it.
        scratch16 = workp.tile([1, 16], F32, tag="scr16", name="scratch16")
        nc.scalar.activation(
            scratch16[:], warm_rhs[0:1, 0:16],
            mybir.ActivationFunctionType.Exp, scale=1.0)

        # st pool first on the left stack (outlives the qk pool)
        st_cm = tc.tile_pool(name="ps_st", bufs=2, space="PSUM")
        st_pool = st_cm.__enter__()
        qk_cm = tc.tile_pool(name="ps_qk", bufs=2, space="PSUM")
        qk_pool = qk_cm.__enter__()

        tr_cm = tc.tile_pool(name="ps_tr", bufs=2, space="PSUM", side="right")
        tr_pool = tr_cm.__enter__()

        def emit_transpose(i):
            # x PE transpose; xT_all[:, cc*T + i*128 : ...] gets tile (i, cc)
            ps_tr = tr_pool.tile([P, C], BF16, tag="tr", name="ps_tr")
            for cc in range(NCT):
                nc.tensor.transpose(
                    ps_tr[:, cc * P:(cc + 1) * P],
                    x_in[:, i * C + cc * P: i * C + (cc + 1) * P],
                    identity[:],
                )
            # one strided copy: dest [128, cc, 128] with cc-stride T
            dst = xT_all[:].rearrange("p (cc t) -> p cc t", cc=NCT)
            nc.vector.tensor_copy(
                dst[:, :, i * P:(i + 1) * P],
                ps_tr[:].rearrange("p (cc q) -> p cc q", cc=NCT),
            )

        def emit_qk(m, j):
            ps_qk = qk_pool.tile([P, CHUNK], F32, tag="qk", name="ps_qk")
            js = slice(j * CHUNK, (j + 1) * CHUNK)
            for cc in range(NCT):
                nc.tensor.matmul(
                    ps_qk[:],
                    wq[cc][:, m * P:(m + 1) * P],
                    xT_all[:, cc * T + j * CHUNK: cc * T + (j + 1) * CHUNK],
                    start=(cc == 0),
                    stop=(cc == NCT - 1),
                )
            dstt = qkT[m] if m < NCT else kT2[m - NCT]
            nc.vector.tensor_scalar_add(dstt[:, js], ps_qk[:], qkb_t[m][:])

        def emit_v(i):
            ps_v = qk_pool.tile([P, CHUNK], F32, tag="qk", name="ps_v")
            for cc in range(NCT):
                nc.tensor.matmul(
                    ps_v[:],
                    xT_all[:, cc * T + i * P: cc * T + (i + 1) * P],
                    wq[cc][:, 2 * C:3 * C],
                    start=(cc == 0),
                    stop=(cc == NCT - 1),
                )
            va3 = vaug[i][:, 0:HEADS * VSTRIDE].rearrange(
                "p (h d) -> p h d", d=VSTRIDE)
            nc.vector.tensor_copy(
                va3[:, :, 0:HC],
                ps_v[:].rearrange("p (h d) -> p h d", h=HEADS),
            )

        # prologue part 2: transposes interleaved with pair 0's q/k tiles so
        # the first S^T slot is reachable as early as possible.  Chunk j of
        # qkT[0]/kT2[0] needs x tiles 4j..4j+3 transposed.
        for i in range(NT // 2):
            emit_transpose(i)
        emit_qk(0, 0)
        emit_qk(NCT, 0)
        for i in range(NT // 2, NT):
            emit_transpose(i)
        emit_qk(0, 1)
        emit_qk(NCT, 1)
        tr_cm.__exit__(None, None, None)

        # filler iterator: remaining phase-1 work in dependency-safe order
        def phase1_fillers():
            for m in [1, NCT + 1, 2, NCT + 2, 3, NCT + 3]:
                for j in range(NCH):
                    yield ("qk", m, j)
            for i in range(NT):
                yield ("v", i)

        fillers = phase1_fillers()
        state = {"fill_done": False, "qk_cm": qk_cm, "pv_pool": None}

        def run_fillers(n):
            for _ in range(n):
                try:
                    f = next(fillers)
                except StopIteration:
                    state["fill_done"] = True
                    return
                if f[0] == "qk":
                    emit_qk(f[1], f[2])
                else:
                    emit_v(f[1])

        # ================= phase 2: attention =================
        def emit_pv_chain(h, j, ppv, exh):
            base = (h % 2) * T
            for ssi in range(NT):
                nc.tensor.matmul(
                    ppv[:],
                    vaug[ssi][:, h * VSTRIDE: h * VSTRIDE + P],
                    exh[:, ssi * 2 * T + base + j * CHUNK:
                        ssi * 2 * T + base + (j + 1) * CHUNK],
                    start=(ssi == 0),
                    stop=(ssi == NT - 1),
                )

        def emit_normalize(h, ppv0, ppv1):
            m = h // 2
            rlo = (h % 2) * HC
            dtmp = workp.tile([1, T], F32, tag="dtmp", name="dtmp")
            nc.vector.tensor_copy(dtmp[:, 0:CHUNK], ppv0[HC:HC + 1, :])
            nc.vector.tensor_copy(dtmp[:, CHUNK:T], ppv1[HC:HC + 1, :])
            recip = workp.tile([1, T], F32, tag="recip", name="recip")
            nc.vector.reciprocal_approx_fast(recip[:], dtmp[:])
            bcast = workp.tile([HC, T], F32, tag="bcast", name="bcast")
            nc.gpsimd.partition_broadcast(bcast[:], recip[:], channels=HC)
            for j, ppv in ((0, ppv0), (1, ppv1)):
                nc.vector.tensor_tensor(
                    anorm[m][rlo:rlo + HC, j * CHUNK:(j + 1) * CHUNK],
                    ppv[0:HC, :],
                    bcast[:, j * CHUNK:(j + 1) * CHUNK],
                    op=mybir.AluOpType.mult,
                )

        def make_pv_steps(p, exh):
            """Six lazy closures: (j0 chain, j1 chain, normalize) x 2 heads.
            Each chain gets its own dedicated PSUM bank (tags pv0..pv3) so a
            chain never WAR-stalls behind the other head's normalize."""
            steps = []
            for hh in range(2):
                h = 2 * p + hh
                box = {}

                def s_j0(h=h, hh=hh, box=box):
                    if state["pv_pool"] is None:
                        # opened lazily AFTER the qk pool closes (PSUM budget)
                        state["pv_cm"] = tc.tile_pool(
                            name="ps_pv", bufs=1, space="PSUM", side="right")
                        state["pv_pool"] = state["pv_cm"].__enter__()
                    box["p0"] = state["pv_pool"].tile(
                        [P, CHUNK], F32, tag=f"pv{2 * hh}", name="ppv0")
                    emit_pv_chain(h, 0, box["p0"], exh)

                def s_j1(h=h, hh=hh, box=box):
                    box["p1"] = state["pv_pool"].tile(
                        [P, CHUNK], F32, tag=f"pv{2 * hh + 1}", name="ppv1")
                    emit_pv_chain(h, 1, box["p1"], exh)

                def s_norm(h=h, box=box):
                    emit_normalize(h, box["p0"], box["p1"])

                steps += [s_j0, s_j1, s_norm]
            return steps

        exh_pool = ctx.enter_context(tc.tile_pool(name="exh", bufs=2))

        # slot list across all pairs; S^T matmuls are emitted ONE SLOT AHEAD
        # of their exp so the ACT-critical feed sits in front of any payload
        # stalls in the PE FIFO.
        slots = [(p, ssi) for p in range(NPAIR) for ssi in range(NT)]
        exhs = []
        st_q = []

        def ensure_exh(p):
            while len(exhs) <= p:
                exhs.append(exh_pool.tile(
                    [P, NT * 2 * T], BF16, tag="exh", name="exh"))

        def emit_st(p, ssi):
            ensure_exh(p)
            sta = st_pool.tile([P, T], F32, tag="st", name="sta")
            stb = st_pool.tile([P, T], F32, tag="st", name="stb")
            for j in range(NCH):
                js = slice(j * CHUNK, (j + 1) * CHUNK)
                nc.tensor.matmul(
                    sta[:, js],
                    kT2[p][0:HC, ssi * P:(ssi + 1) * P],
                    qkT[p][0:HC, js],
                    start=True, stop=True,
                )
                nc.tensor.matmul(
                    stb[:, js],
                    kT2[p][HC:P, ssi * P:(ssi + 1) * P],
                    qkT[p][HC:P, js],
                    start=True, stop=True,
                )
            st_q.append((sta, stb))

        pending = []  # queue of PV/normalize closures for the previous pair
        emit_st(*slots[0])
        for g, (p, ssi) in enumerate(slots):
            exh = exhs[p]
            sta, stb = st_q.pop(0)
            nc.scalar.activation(
                exh[:, ssi * 2 * T: ssi * 2 * T + T],
                sta[:],
                mybir.ActivationFunctionType.Exp,
                scale=EXP_SCALE,
            )
            nc.scalar.activation(
                exh[:, ssi * 2 * T + T: (ssi + 1) * 2 * T],
                stb[:],
                mybir.ActivationFunctionType.Exp,
                scale=EXP_SCALE,
            )
            # next slot's S^T goes in front of this slot's payload work
            if g + 1 < len(slots):
                emit_st(*slots[g + 1])
            if ssi == 0 and p > 0:
                pending.extend(make_pv_steps(p - 1, exhs[p - 1]))
                if debug_dumps and p == 1:
                    nc.sync.dma_start(
                        dbg["dbg_exh0"].ap(), exhs[0][:, 0:2 * T])
            # PE-gap payloads for this slot
            if not state["fill_done"]:
                run_fillers(2)
            else:
                if state["qk_cm"] is not None:
                    state["qk_cm"].__exit__(None, None, None)
                    state["qk_cm"] = None
                if pending:
                    pending.pop(0)()
        if state["qk_cm"] is not None:
            state["qk_cm"].__exit__(None, None, None)
            state["qk_cm"] = None
        # tail: whatever remains of pair 2's PV, then pair 3's PV + normalize
        pending.extend(make_pv_steps(NPAIR - 1, exhs[-1]))
        while pending:
            pending.pop(0)()
        st_cm.__exit__(None, None, None)
        if state["pv_pool"] is not None:
            state["pv_cm"].__exit__(None, None, None)
            state["pv_pool"] = None

        if debug_dumps:
            nc.sync.dma_start(dbg["dbg_xT"].ap(), xT_all[:])
            nc.sync.dma_start(dbg["dbg_q0"].ap(), qkT[0][:])
            nc.sync.dma_start(dbg["dbg_k0"].ap(), kT2[0][:])
            nc.sync.dma_start(dbg["dbg_va0"].ap(), vaug[0][:])
            nc.sync.dma_start(dbg["dbg_an0"].ap(), anorm[0][:])

        # ================= phase 3: out projection =================
        with tc.tile_pool(name="ps_o", bufs=2, space="PSUM") as ps_op:
            for i in range(NT):
                ps_o = ps_op.tile([P, C], F32, tag="o", name="ps_o")
                for cc in range(NCT):
                    nc.tensor.matmul(
                        ps_o[:],
                        anorm[cc][:, i * P:(i + 1) * P],
                        wo[cc][:],
                        start=(cc == 0),
                        stop=(cc == NCT - 1),
                    )
                osb = workp.tile([P, C], BF16, tag=f"osb{i}", name=f"osb{i}")
                nc.vector.tensor_tensor(
                    osb[:], ps_o[:], ob_bcast[:], op=mybir.AluOpType.add)
                nc.sync.dma_start(out_d.ap()[i * P:(i + 1) * P, :], osb[:])

    nc.compile()
    return nc


_CACHED_NC = None


def _get_nc():
    global _CACHED_NC
    if _CACHED_NC is None:
        _CACHED_NC = build_program(
            debug_dumps=bool(int(os.environ.get("KERNEL_DEBUG", "0"))))
    return _CACHED_NC


def _prep_inputs(x, qkv_w, qkv_b, out_w, out_b):
    import ml_dtypes

    x = np.asarray(x)
    B = x.shape[0]
    x2 = x.reshape(B, T, C).astype(ml_dtypes.bfloat16)
    wq2 = np.asarray(qkv_w).reshape(C, 3 * C).astype(ml_dtypes.bfloat16)
    wo2 = np.asarray(out_w).reshape(C, C).astype(ml_dtypes.bfloat16)
    qkv_b = np.asarray(qkv_b).astype(np.float32)
    out_b = np.asarray(out_b).astype(np.float32)
    # fold the v-bias through the output projection (exact: A_norm += b_v
    # shifts out by b_v @ W_out since softmax rows sum to 1).
    b_v = qkv_b[2 * C:3 * C]
    ob_eff = (
        out_b.astype(np.float64)
        + b_v.astype(np.float64) @ wo2.astype(np.float64)
    ).astype(np.float32)
    qkb = np.ascontiguousarray(qkv_b[0:2 * C])
    return x2, wq2, wo2, qkb, ob_eff


def kernel(x, qkv_w, qkv_b, out_w, out_b):
    """Full inputs in, full output out.  Shards batch across 8 NeuronCores."""
    from concourse.bass_utils import run_bass_kernel_spmd

    x = np.asarray(x)
    B, H, W, Cc = x.shape
    assert (B, H, W, Cc) == (8, 32, 32, C)
    x2, wq2, wo2, qkb, ob_eff = _prep_inputs(x, qkv_w, qkv_b, out_w, out_b)

    nc = _get_nc()
    in_maps = [
        {
            "x": np.ascontiguousarray(x2[b]),
            "qkv_w": np.ascontiguousarray(wq2),
            "out_w": np.ascontiguousarray(wo2),
            "qk_b": qkb,
            "out_b": ob_eff,
        }
        for b in range(B)
    ]
    trace = bool(int(os.environ.get("KERNEL_TRACE", "0")))
    res = run_bass_kernel_spmd(nc, in_maps, core_ids=list(range(B)), trace=trace)
    if trace and res.exec_time_ns is not None:
        print(f"HW exec time: {res.exec_time_ns} ns")
    kernel.last_results = res
    out = np.stack(
        [np.asarray(res.results[b]["out"]).astype(np.float32) for b in range(B)],
        axis=0,
    )
    return out.reshape(B, H, W, Cc)


kernel.last_results = None
